# revision 19
# baseline (speedup 1.0000x reference)
"""Trainium2 Bass kernel for the GNN message-passing layer.

Strategy (pure data-parallel over batch, 8 NeuronCores, 16 batches/core):
  - Host pre-transposes activations to feature-major layout so the device
    does zero on-chip transposes: all compute runs in [feature, element]
    layout with 2 batches packed on the 128 SBUF partitions.
  - Edge update: the (M,K)-grid broadcast-add of the per-m / per-k node
    terms is folded into the TensorE pass as PSUM-accumulated matmuls
    against a 0/1 indicator matrix, so VectorE never touches it.
  - Main matmuls run as bf16 hi/lo split (x = xhi + xlo, W = Whi + Wlo;
    keep Whi*xhi + Whi*xlo + Wlo*xhi): same DMA bytes as fp32, 1 cyc/row
    on the PE, ~1e-6 relative error. (MM_MODE="f32r" trades accuracy
    ~1.6e-4 for ~2x less PE work; "f32" is the 4 cyc/row exact path.)
  - leaky_relu runs on ScalarE (PSUM->SBUF, one pass).
  - mean-over-k runs on VectorE (one strided reduce per pair);
    mean-over-m runs on GpSimd as a log2 tree of tensor adds.
  - Node updates are tiny block-diagonal fp32 matmuls (mean 1/64 folded
    into the host-prepared Wneigh weights).
  - Loads issue on the SP HWDGE queue, stores on the ACT HWDGE queue so
    stores never head-of-line-block the next pair's loads.
"""
import sys

import numpy as np

sys.path.insert(0, "/opt/trn_rl_repo")

BS, M, K, FIN, FOUT, E = 128, 64, 64, 64, 64, 4096
NCORES = 8
BPC = BS // NCORES          # batches per core
PAIRS = BPC // 2            # 2 batches packed per 128 partitions
NEG_SLOPE = 0.01
WAVE = 1024                 # psum wave width (2 banks)
WAVES = E // WAVE

MM_MODE = "bf16split"       # "bf16split" | "f32r" | "f32"
NK_ON_GPSIMD = False        # Sum-over-m tree on GpSimd (else VectorE)

_CACHE = {}


def _build_nc(mm_mode=MM_MODE, nk_on_gpsimd=NK_ON_GPSIMD):
    import concourse.bacc as bacc
    import concourse.tile as tile
    from concourse import mybir
    from contextlib import ExitStack

    f32 = mybir.dt.float32
    bf16 = mybir.dt.bfloat16
    fmm = {"bf16split": bf16, "f32r": mybir.dt.float32r, "f32": f32}[mm_mode]
    split = mm_mode == "bf16split"
    Lrelu = mybir.ActivationFunctionType.Lrelu
    add = mybir.AluOpType.add
    X = mybir.AxisListType.X

    nc = bacc.Bacc("TRN2", target_bir_lowering=False, debug=False)

    # DRAM I/O (per core shard)
    xhi_d = nc.dram_tensor("xhi", [BPC, FIN, E], fmm, kind="ExternalInput").ap()
    if split:
        xlo_d = nc.dram_tensor("xlo", [BPC, FIN, E], fmm, kind="ExternalInput").ap()
    zmt_d = nc.dram_tensor("zmt", [BPC, FIN, M], f32, kind="ExternalInput").ap()
    zkt_d = nc.dram_tensor("zkt", [BPC, FIN, K], f32, kind="ExternalInput").ap()
    whi_d = nc.dram_tensor("wedgehi_bd", [128, 128], fmm, kind="ExternalInput").ap()
    if split:
        wlo_d = nc.dram_tensor("wedgelo_bd", [128, 128], fmm, kind="ExternalInput").ap()
    wm_d = nc.dram_tensor("wm_bd", [128, 128], f32, kind="ExternalInput").ap()
    wk_d = nc.dram_tensor("wk_bd", [128, 128], f32, kind="ExternalInput").ap()
    wsm_d = nc.dram_tensor("wselfm_bd", [128, 128], f32, kind="ExternalInput").ap()
    wsk_d = nc.dram_tensor("wselfk_bd", [128, 128], f32, kind="ExternalInput").ap()
    wnm_d = nc.dram_tensor("wneighm_bd", [128, 128], f32, kind="ExternalInput").ap()
    wnk_d = nc.dram_tensor("wneighk_bd", [128, 128], f32, kind="ExternalInput").ap()
    eye_d = nc.dram_tensor("eye64", [64, 64], fmm, kind="ExternalInput").ap()

    edges_o = nc.dram_tensor("edges_t", [BPC, FOUT, E], f32, kind="ExternalOutput").ap()
    zmup_o = nc.dram_tensor("zmup_t", [BPC, FOUT, M], f32, kind="ExternalOutput").ap()
    zkup_o = nc.dram_tensor("zkup_t", [BPC, FOUT, K], f32, kind="ExternalOutput").ap()

    ld = nc.sync          # load queue (SP HWDGE)
    st = nc.scalar        # store queue (ACT HWDGE)

    with tile.TileContext(nc) as tc, ExitStack() as ctx:
        consts = ctx.enter_context(tc.tile_pool(name="consts", bufs=1))
        xts = ctx.enter_context(tc.tile_pool(name="xts", bufs=3))
        edges_pool = ctx.enter_context(tc.tile_pool(name="edges", bufs=3))
        small_in = ctx.enter_context(tc.tile_pool(name="small_in", bufs=3))
        work = ctx.enter_context(tc.tile_pool(name="work", bufs=3))
        psum_main = ctx.enter_context(tc.tile_pool(name="psmain", bufs=2, space="PSUM"))
        psum_small = ctx.enter_context(tc.tile_pool(name="pssmall", bufs=4, space="PSUM"))

        # ---- constants ----
        def load_const(name, ap_d, shape, dt=f32):
            t = consts.tile(shape, dt, tag=name)
            ld.dma_start(out=t[:], in_=ap_d)
            return t

        whi_sb = load_const("whi", whi_d, [128, 128], dt=fmm)
        if split:
            wlo_sb = load_const("wlo", wlo_d, [128, 128], dt=fmm)
        wm_sb = load_const("wm", wm_d, [128, 128])
        wk_sb = load_const("wk", wk_d, [128, 128])
        wsm_sb = load_const("wsm", wsm_d, [128, 128])
        wsk_sb = load_const("wsk", wsk_d, [128, 128])
        wnm_sb = load_const("wnm", wnm_d, [128, 128])
        wnk_sb = load_const("wnk", wnk_d, [128, 128])
        eye_sb = load_const("eye", eye_d, [64, 64], dt=fmm)

        # indicator [128, E]: rows 0-63 select m(i)=i//64, rows 64-127 select k(i)=i%64
        ind_sb = consts.tile([128, E], fmm, tag="ind")
        nc.vector.tensor_copy(
            ind_sb[0:64, :].rearrange("p (m k) -> p m k", k=K),
            eye_sb[:, :, None].to_broadcast([64, M, K]),
        )
        nc.vector.tensor_copy(
            ind_sb[64:128, :].rearrange("p (m k) -> p m k", k=K),
            eye_sb[:, None, :].to_broadcast([64, M, K]),
        )

        for p in range(PAIRS):
            b0 = 2 * p
            xhi_pair = xts.tile([128, E], fmm, tag="xhi_pair")
            ld.dma_start(out=xhi_pair[:], in_=xhi_d[b0:b0 + 2].rearrange("b f e -> (b f) e"))
            if split:
                xlo_pair = xts.tile([128, E], fmm, tag="xlo_pair")
                ld.dma_start(out=xlo_pair[:], in_=xlo_d[b0:b0 + 2].rearrange("b f e -> (b f) e"))
            zmt_pair = small_in.tile([128, M], f32, tag="zmt_pair")
            ld.dma_start(out=zmt_pair[:], in_=zmt_d[b0:b0 + 2].rearrange("b f m -> (b f) m"))
            zkt_pair = small_in.tile([128, K], f32, tag="zkt_pair")
            ld.dma_start(out=zkt_pair[:], in_=zkt_d[b0:b0 + 2].rearrange("b f k -> (b f) k"))

            # Wz_m / Wz_k in [node, (b, o)] layout -> stacked as lhsT for the S-matmul
            wzm_ps = psum_small.tile([64, 128], f32, tag="sm")
            nc.tensor.matmul(wzm_ps[:], zmt_pair[:], wm_sb[:], start=True, stop=True)
            wzk_ps = psum_small.tile([64, 128], f32, tag="sm")
            nc.tensor.matmul(wzk_ps[:], zkt_pair[:], wk_sb[:], start=True, stop=True)
            s_hi = work.tile([128, 128], fmm, tag="s_hi")
            nc.scalar.copy(out=s_hi[0:64, :], in_=wzm_ps[:])
            nc.scalar.copy(out=s_hi[64:128, :], in_=wzk_ps[:])
            if split:
                s_lo = work.tile([128, 128], fmm, tag="s_lo")
                nc.vector.tensor_tensor(
                    s_lo[0:64, :], wzm_ps[:], s_hi[0:64, :], mybir.AluOpType.subtract)
                nc.vector.tensor_tensor(
                    s_lo[64:128, :], wzk_ps[:], s_hi[64:128, :], mybir.AluOpType.subtract)

            edges_sb = edges_pool.tile([128, E], f32, tag="edges_sb")
            sum_nm = work.tile([128, M], f32, tag="sum_nm")
            nk_acc = work.tile([128, WAVE], f32, tag="nk_acc")

            for w in range(WAVES):
                ps = psum_main.tile([128, WAVE], f32, tag="ps")

                # Quadrant-tiled matmuls: per 512-col chunk the S terms run as a
                # row-split concurrent pair (K=64 each) and each main term as a
                # batch-split concurrent pair on disjoint (row, col) quadrants,
                # so LDWEIGHTS of one quadrant hides under the other's MATMUL.
                for h in range(2):
                    pc = ps[:, h * 512:(h + 1) * 512]
                    sl = slice(w * WAVE + h * 512, w * WAVE + (h + 1) * 512)

                    # Full-K block-diagonal matmuls (5 per chunk), one
                    # accumulation group per 512-col psum bank.
                    s_terms = [s_hi, s_lo] if split else [s_hi]
                    main_terms = [(whi_sb, xhi_pair)]
                    if split:
                        main_terms += [(whi_sb, xlo_pair), (wlo_sb, xhi_pair)]
                    for si, s_sb in enumerate(s_terms):
                        nc.tensor.matmul(pc, s_sb[:], ind_sb[:, sl],
                                         start=(si == 0), stop=False)
                    for ti, (wsb, xsb) in enumerate(main_terms):
                        nc.tensor.matmul(pc, wsb[:], xsb[:, sl],
                                         start=False, stop=(ti == len(main_terms) - 1))

                wsl = slice(w * WAVE, (w + 1) * WAVE)
                nc.scalar.activation(
                    out=edges_sb[:, wsl], in_=ps[:], func=Lrelu, alpha=NEG_SLOPE,
                )
                # incremental reductions per wave
                mpw = WAVE // K
                nc.vector.tensor_reduce(
                    out=sum_nm[:, w * mpw:(w + 1) * mpw],
                    in_=edges_sb[:, wsl].rearrange("p (m k) -> p m k", k=K),
                    axis=X, op=add,
                )
                if nk_on_gpsimd:
                    if w == 0:
                        nc.gpsimd.tensor_copy(nk_acc[:], edges_sb[:, wsl])
                    else:
                        nc.gpsimd.tensor_tensor(nk_acc[:], nk_acc[:],
                                                edges_sb[:, wsl], add)

            sum_nk = work.tile([128, K], f32, tag="sum_nk")
            if nk_on_gpsimd:
                # tree-fold the 16 m's left in nk_acc
                for width in (512, 256, 128, 64):
                    nc.gpsimd.tensor_tensor(nk_acc[:, 0:width], nk_acc[:, 0:width],
                                            nk_acc[:, width:2 * width], add)
                nc.gpsimd.tensor_copy(sum_nk[:], nk_acc[:, 0:64])
            else:
                nc.vector.tensor_reduce(
                    out=sum_nk[:],
                    in_=edges_sb[:].rearrange("p (m k) -> p k m", k=K),
                    axis=X, op=add,
                )

            st.dma_start(
                out=edges_o[b0:b0 + 2].rearrange("b o e -> (b o) e"), in_=edges_sb[:]
            )

            # node updates
            node_m_ps = psum_small.tile([128, M], f32, tag="sm")
            nc.tensor.matmul(node_m_ps[:], wsm_sb[:], zmt_pair[:], start=True, stop=False)
            nc.tensor.matmul(node_m_ps[:], wnm_sb[:], sum_nm[:], start=False, stop=True)
            node_k_ps = psum_small.tile([128, K], f32, tag="sm")
            nc.tensor.matmul(node_k_ps[:], wsk_sb[:], zkt_pair[:], start=True, stop=False)
            nc.tensor.matmul(node_k_ps[:], wnk_sb[:], sum_nk[:], start=False, stop=True)

            node_m_sb = work.tile([128, M], f32, tag="node_m_sb")
            nc.scalar.activation(out=node_m_sb[:], in_=node_m_ps[:], func=Lrelu, alpha=NEG_SLOPE)
            node_k_sb = work.tile([128, K], f32, tag="node_k_sb")
            nc.scalar.activation(out=node_k_sb[:], in_=node_k_ps[:], func=Lrelu, alpha=NEG_SLOPE)

            st.dma_start(out=zmup_o[b0:b0 + 2].rearrange("b o m -> (b o) m"), in_=node_m_sb[:])
            st.dma_start(out=zkup_o[b0:b0 + 2].rearrange("b o k -> (b o) k"), in_=node_k_sb[:])

    nc.compile()
    return nc


def _blockdiag(w):
    out = np.zeros((128, 128), w.dtype)
    out[:64, :64] = w
    out[64:, 64:] = w
    return out


def _host_prep(inputs, mm_mode=MM_MODE):
    import ml_dtypes

    bf16 = ml_dtypes.bfloat16
    split = mm_mode == "bf16split"
    npmm = bf16 if split else np.float32

    z_mk = np.asarray(inputs["z_mk"], np.float32)
    z_m = np.asarray(inputs["z_m"], np.float32)
    z_k = np.asarray(inputs["z_k"], np.float32)
    xt = np.ascontiguousarray(z_mk.transpose(0, 2, 1))
    zmt = np.ascontiguousarray(z_m.transpose(0, 2, 1))
    zkt = np.ascontiguousarray(z_k.transpose(0, 2, 1))

    wedge_t = np.asarray(inputs["Wedge"], np.float32).T
    if split:
        xhi = xt.astype(bf16)
        xlo = (xt - xhi.astype(np.float32)).astype(bf16)
        whi = wedge_t.astype(bf16)
        wlo = (wedge_t - whi.astype(np.float32)).astype(bf16)
    else:
        xhi, xlo = xt, None
        whi, wlo = wedge_t, None

    consts = {
        "wedgehi_bd": _blockdiag(np.ascontiguousarray(whi.astype(npmm))),
        "wm_bd": _blockdiag(np.asarray(inputs["Wm"], np.float32).T),
        "wk_bd": _blockdiag(np.asarray(inputs["Wk"], np.float32).T),
        "wselfm_bd": _blockdiag(np.asarray(inputs["Wself_m"], np.float32).T),
        "wselfk_bd": _blockdiag(np.asarray(inputs["Wself_k"], np.float32).T),
        "wneighm_bd": _blockdiag(np.asarray(inputs["Wneigh_m"], np.float32).T / 64.0),
        "wneighk_bd": _blockdiag(np.asarray(inputs["Wneigh_k"], np.float32).T / 64.0),
        "eye64": np.eye(64).astype(npmm),
    }
    if split:
        consts["wedgelo_bd"] = _blockdiag(np.ascontiguousarray(wlo))

    in_maps = []
    for c in range(NCORES):
        sl = slice(c * BPC, (c + 1) * BPC)
        m = {"xhi": np.ascontiguousarray(xhi[sl]),
             "zmt": np.ascontiguousarray(zmt[sl]),
             "zkt": np.ascontiguousarray(zkt[sl])}
        if split:
            m["xlo"] = np.ascontiguousarray(xlo[sl])
        m.update(consts)
        in_maps.append(m)
    return in_maps


def _postprocess(results):
    edges_t = np.concatenate([r["edges_t"] for r in results], axis=0)
    zmup_t = np.concatenate([r["zmup_t"] for r in results], axis=0)
    zkup_t = np.concatenate([r["zkup_t"] for r in results], axis=0)
    z_mk_updated = np.ascontiguousarray(edges_t.transpose(0, 2, 1)).reshape(BS, M * K, FOUT)
    z_m_updated = np.ascontiguousarray(zmup_t.transpose(0, 2, 1))
    z_k_updated = np.ascontiguousarray(zkup_t.transpose(0, 2, 1))
    return z_mk_updated, z_m_updated, z_k_updated


def run_on_hw(in_maps, trace=False, **kwargs):
    from concourse.bass_utils import run_bass_kernel_spmd

    if "nc" not in _CACHE:
        _CACHE["nc"] = _build_nc()
    res = run_bass_kernel_spmd(
        _CACHE["nc"], in_maps, core_ids=list(range(NCORES)), trace=trace, **kwargs
    )
    return res


def kernel(**inputs):
    in_maps = _host_prep(inputs)
    res = run_on_hw(in_maps)
    return _postprocess(res.results)


# revision 23
# speedup vs baseline: 1.1269x; 1.1269x over previous
"""Trainium2 Bass kernel for the GNN message-passing layer.

Strategy (pure data-parallel over batch, 8 NeuronCores, 16 batches/core):
  - Host pre-transposes activations to feature-major layout so the device
    does zero on-chip transposes: all compute runs in [feature, element]
    layout with 2 batches packed on the 128 SBUF partitions.
  - Edge update: the (M,K)-grid broadcast-add of the per-m / per-k node
    terms is folded into the TensorE pass as PSUM-accumulated matmuls
    against a 0/1 indicator matrix, so VectorE never touches it.
  - Main matmuls run as bf16 hi/lo split (x = xhi + xlo, W = Whi + Wlo;
    keep Whi*xhi + Whi*xlo + Wlo*xhi): same DMA bytes as fp32, 1 cyc/row
    on the PE, ~1e-6 relative error. (MM_MODE="f32r" trades accuracy
    ~1.6e-4 for ~2x less PE work; "f32" is the 4 cyc/row exact path.)
  - leaky_relu runs on ScalarE (PSUM->SBUF, one pass).
  - mean-over-k runs on VectorE (one strided reduce per pair);
    mean-over-m runs on GpSimd as a log2 tree of tensor adds.
  - Node updates are tiny block-diagonal fp32 matmuls (mean 1/64 folded
    into the host-prepared Wneigh weights).
  - Loads issue on the SP HWDGE queue, stores on the ACT HWDGE queue so
    stores never head-of-line-block the next pair's loads.
"""
import sys

import numpy as np

sys.path.insert(0, "/opt/trn_rl_repo")

BS, M, K, FIN, FOUT, E = 128, 64, 64, 64, 64, 4096
NCORES = 8
BPC = BS // NCORES          # batches per core
PAIRS = BPC // 2            # 2 batches packed per 128 partitions
NEG_SLOPE = 0.01
WAVE = 1024                 # psum wave width (2 banks)
WAVES = E // WAVE

MM_MODE = "bf16split"       # "bf16split" | "f32r" | "f32"
NK_ON_GPSIMD = False        # Sum-over-m tree on GpSimd (else VectorE)

_CACHE = {}


def _build_nc(mm_mode=MM_MODE, nk_on_gpsimd=NK_ON_GPSIMD):
    import concourse.bacc as bacc
    import concourse.tile as tile
    from concourse import mybir
    from contextlib import ExitStack

    f32 = mybir.dt.float32
    bf16 = mybir.dt.bfloat16
    fmm = {"bf16split": bf16, "f32r": mybir.dt.float32r, "f32": f32}[mm_mode]
    split = mm_mode == "bf16split"
    Lrelu = mybir.ActivationFunctionType.Lrelu
    add = mybir.AluOpType.add
    X = mybir.AxisListType.X

    nc = bacc.Bacc("TRN2", target_bir_lowering=False, debug=False)

    # DRAM I/O (per core shard)
    xhi_d = nc.dram_tensor("xhi", [BPC, FIN, E], fmm, kind="ExternalInput").ap()
    if split:
        xlo_d = nc.dram_tensor("xlo", [BPC, FIN, E], fmm, kind="ExternalInput").ap()
    zmt_d = nc.dram_tensor("zmt", [BPC, FIN, M], f32, kind="ExternalInput").ap()
    zkt_d = nc.dram_tensor("zkt", [BPC, FIN, K], f32, kind="ExternalInput").ap()
    whi_d = nc.dram_tensor("wedgehi_bd", [128, 128], fmm, kind="ExternalInput").ap()
    if split:
        wlo_d = nc.dram_tensor("wedgelo_bd", [128, 128], fmm, kind="ExternalInput").ap()
    wm_d = nc.dram_tensor("wm_bd", [128, 128], f32, kind="ExternalInput").ap()
    wk_d = nc.dram_tensor("wk_bd", [128, 128], f32, kind="ExternalInput").ap()
    wsm_d = nc.dram_tensor("wselfm_bd", [128, 128], f32, kind="ExternalInput").ap()
    wsk_d = nc.dram_tensor("wselfk_bd", [128, 128], f32, kind="ExternalInput").ap()
    wnm_d = nc.dram_tensor("wneighm_bd", [128, 128], f32, kind="ExternalInput").ap()
    wnk_d = nc.dram_tensor("wneighk_bd", [128, 128], f32, kind="ExternalInput").ap()
    eye_d = nc.dram_tensor("eye64", [64, 64], fmm, kind="ExternalInput").ap()

    edges_o = nc.dram_tensor("edges_t", [BPC, FOUT, E], f32, kind="ExternalOutput").ap()
    zmup_o = nc.dram_tensor("zmup_t", [BPC, FOUT, M], f32, kind="ExternalOutput").ap()
    zkup_o = nc.dram_tensor("zkup_t", [BPC, FOUT, K], f32, kind="ExternalOutput").ap()

    ld = nc.sync          # load queue (SP HWDGE)
    st = nc.scalar        # store queue (ACT HWDGE)

    with tile.TileContext(nc) as tc, ExitStack() as ctx:
        consts = ctx.enter_context(tc.tile_pool(name="consts", bufs=1))
        xts = ctx.enter_context(tc.tile_pool(name="xts", bufs=3))
        edges_pool = ctx.enter_context(tc.tile_pool(name="edges", bufs=3))
        small_in = ctx.enter_context(tc.tile_pool(name="small_in", bufs=3))
        work = ctx.enter_context(tc.tile_pool(name="work", bufs=3))
        psum_main = ctx.enter_context(tc.tile_pool(name="psmain", bufs=2, space="PSUM"))
        psum_wz = ctx.enter_context(tc.tile_pool(name="pswz", bufs=2, space="PSUM"))
        psum_node = ctx.enter_context(tc.tile_pool(name="psnode", bufs=2, space="PSUM"))

        # ---- constants ----
        def load_const(name, ap_d, shape, dt=f32):
            t = consts.tile(shape, dt, tag=name)
            ld.dma_start(out=t[:], in_=ap_d)
            return t

        whi_sb = load_const("whi", whi_d, [128, 128], dt=fmm)
        if split:
            wlo_sb = load_const("wlo", wlo_d, [128, 128], dt=fmm)
        wm_sb = load_const("wm", wm_d, [128, 128])
        wk_sb = load_const("wk", wk_d, [128, 128])
        wsm_sb = load_const("wsm", wsm_d, [128, 128])
        wsk_sb = load_const("wsk", wsk_d, [128, 128])
        wnm_sb = load_const("wnm", wnm_d, [128, 128])
        wnk_sb = load_const("wnk", wnk_d, [128, 128])
        eye_sb = load_const("eye", eye_d, [64, 64], dt=fmm)

        # indicator [128, E]: rows 0-63 select m(i)=i//64, rows 64-127 select k(i)=i%64
        ind_sb = consts.tile([128, E], fmm, tag="ind")
        nc.vector.tensor_copy(
            ind_sb[0:64, :].rearrange("p (m k) -> p m k", k=K),
            eye_sb[:, :, None].to_broadcast([64, M, K]),
        )
        nc.vector.tensor_copy(
            ind_sb[64:128, :].rearrange("p (m k) -> p m k", k=K),
            eye_sb[:, None, :].to_broadcast([64, M, K]),
        )

        def node_updates(prev):
            """Deferred node-update block for a finished pair (runs while the
            next pair's waves occupy the PE, so the sum_nk dependency never
            head-of-line-blocks the wave matmuls)."""
            b0, zmt_pair, zkt_pair, sum_nm, sum_nk = prev
            node_m_ps = psum_node.tile([128, M], f32, tag="nd")
            nc.tensor.matmul(node_m_ps[:], wsm_sb[:], zmt_pair[:], start=True, stop=False)
            nc.tensor.matmul(node_m_ps[:], wnm_sb[:], sum_nm[:], start=False, stop=True)
            node_k_ps = psum_node.tile([128, K], f32, tag="nd")
            nc.tensor.matmul(node_k_ps[:], wsk_sb[:], zkt_pair[:], start=True, stop=False)
            nc.tensor.matmul(node_k_ps[:], wnk_sb[:], sum_nk[:], start=False, stop=True)
            node_m_sb = work.tile([128, M], f32, tag="node_m_sb")
            nc.scalar.activation(out=node_m_sb[:], in_=node_m_ps[:], func=Lrelu, alpha=NEG_SLOPE)
            node_k_sb = work.tile([128, K], f32, tag="node_k_sb")
            nc.scalar.activation(out=node_k_sb[:], in_=node_k_ps[:], func=Lrelu, alpha=NEG_SLOPE)
            st.dma_start(out=zmup_o[b0:b0 + 2].rearrange("b o m -> (b o) m"), in_=node_m_sb[:])
            st.dma_start(out=zkup_o[b0:b0 + 2].rearrange("b o k -> (b o) k"), in_=node_k_sb[:])

        prev_pair = None
        for p in range(PAIRS):
            b0 = 2 * p
            xhi_pair = xts.tile([128, E], fmm, tag="xhi_pair")
            ld.dma_start(out=xhi_pair[:], in_=xhi_d[b0:b0 + 2].rearrange("b f e -> (b f) e"))
            if split:
                xlo_pair = xts.tile([128, E], fmm, tag="xlo_pair")
                ld.dma_start(out=xlo_pair[:], in_=xlo_d[b0:b0 + 2].rearrange("b f e -> (b f) e"))
            zmt_pair = small_in.tile([128, M], f32, tag="zmt_pair")
            ld.dma_start(out=zmt_pair[:], in_=zmt_d[b0:b0 + 2].rearrange("b f m -> (b f) m"))
            zkt_pair = small_in.tile([128, K], f32, tag="zkt_pair")
            ld.dma_start(out=zkt_pair[:], in_=zkt_d[b0:b0 + 2].rearrange("b f k -> (b f) k"))

            # Wz_m / Wz_k in [node, (b, o)] layout -> stacked as lhsT for the S-matmul
            wzm_ps = psum_wz.tile([64, 128], f32, tag="wz")
            nc.tensor.matmul(wzm_ps[:], zmt_pair[:], wm_sb[:], start=True, stop=True)
            wzk_ps = psum_wz.tile([64, 128], f32, tag="wz")
            nc.tensor.matmul(wzk_ps[:], zkt_pair[:], wk_sb[:], start=True, stop=True)
            s_hi = work.tile([128, 128], fmm, tag="s_hi")
            nc.scalar.copy(out=s_hi[0:64, :], in_=wzm_ps[:])
            nc.scalar.copy(out=s_hi[64:128, :], in_=wzk_ps[:])
            if split:
                s_lo = work.tile([128, 128], fmm, tag="s_lo")
                nc.vector.tensor_tensor(
                    s_lo[0:64, :], wzm_ps[:], s_hi[0:64, :], mybir.AluOpType.subtract)
                nc.vector.tensor_tensor(
                    s_lo[64:128, :], wzk_ps[:], s_hi[64:128, :], mybir.AluOpType.subtract)

            edges_sb = edges_pool.tile([128, E], f32, tag="edges_sb")
            sum_nm = work.tile([128, M], f32, tag="sum_nm")
            nk_parts = work.tile([128, WAVES, K], f32, tag="nk_parts")

            for w in range(WAVES):
                ps = psum_main.tile([128, WAVE], f32, tag="ps")

                # Quadrant-tiled matmuls: per 512-col chunk the S terms run as a
                # row-split concurrent pair (K=64 each) and each main term as a
                # batch-split concurrent pair on disjoint (row, col) quadrants,
                # so LDWEIGHTS of one quadrant hides under the other's MATMUL.
                for h in range(2):
                    pc = ps[:, h * 512:(h + 1) * 512]
                    sl = slice(w * WAVE + h * 512, w * WAVE + (h + 1) * 512)

                    # Full-K block-diagonal matmuls (5 per chunk), one
                    # accumulation group per 512-col psum bank.
                    s_terms = [s_hi, s_lo] if split else [s_hi]
                    main_terms = [(whi_sb, xhi_pair)]
                    if split:
                        main_terms += [(whi_sb, xlo_pair), (wlo_sb, xhi_pair)]
                    for si, s_sb in enumerate(s_terms):
                        nc.tensor.matmul(pc, s_sb[:], ind_sb[:, sl],
                                         start=(si == 0), stop=False)
                    for ti, (wsb, xsb) in enumerate(main_terms):
                        nc.tensor.matmul(pc, wsb[:], xsb[:, sl],
                                         start=False, stop=(ti == len(main_terms) - 1))

                wsl = slice(w * WAVE, (w + 1) * WAVE)
                nc.scalar.activation(
                    out=edges_sb[:, wsl], in_=ps[:], func=Lrelu, alpha=NEG_SLOPE,
                )
                # incremental reductions per wave (hidden under the next
                # wave's matmuls)
                mpw = WAVE // K
                nc.vector.tensor_reduce(
                    out=sum_nm[:, w * mpw:(w + 1) * mpw],
                    in_=edges_sb[:, wsl].rearrange("p (m k) -> p m k", k=K),
                    axis=X, op=add,
                )
                nc.vector.tensor_reduce(
                    out=nk_parts[:, w, :],
                    in_=edges_sb[:, wsl].rearrange("p (m k) -> p k m", k=K),
                    axis=X, op=add,
                )
                # the previous pair's node updates slot in after wave 0 so
                # they sit behind only one wave group in the PE queue
                if w == 0 and prev_pair is not None:
                    node_updates(prev_pair)

            sum_nk = work.tile([128, K], f32, tag="sum_nk")
            nc.vector.tensor_reduce(
                out=sum_nk[:],
                in_=nk_parts[:].rearrange("p w k -> p k w"),
                axis=X, op=add,
            )

            st.dma_start(
                out=edges_o[b0:b0 + 2].rearrange("b o e -> (b o) e"), in_=edges_sb[:]
            )
            prev_pair = (b0, zmt_pair, zkt_pair, sum_nm, sum_nk)

        node_updates(prev_pair)

    nc.compile()
    return nc


def _blockdiag(w):
    out = np.zeros((128, 128), w.dtype)
    out[:64, :64] = w
    out[64:, 64:] = w
    return out


def _host_prep(inputs, mm_mode=MM_MODE):
    import ml_dtypes

    bf16 = ml_dtypes.bfloat16
    split = mm_mode == "bf16split"
    npmm = bf16 if split else np.float32

    z_mk = np.asarray(inputs["z_mk"], np.float32)
    z_m = np.asarray(inputs["z_m"], np.float32)
    z_k = np.asarray(inputs["z_k"], np.float32)
    xt = np.ascontiguousarray(z_mk.transpose(0, 2, 1))
    zmt = np.ascontiguousarray(z_m.transpose(0, 2, 1))
    zkt = np.ascontiguousarray(z_k.transpose(0, 2, 1))

    wedge_t = np.asarray(inputs["Wedge"], np.float32).T
    if split:
        xhi = xt.astype(bf16)
        xlo = (xt - xhi.astype(np.float32)).astype(bf16)
        whi = wedge_t.astype(bf16)
        wlo = (wedge_t - whi.astype(np.float32)).astype(bf16)
    else:
        xhi, xlo = xt, None
        whi, wlo = wedge_t, None

    consts = {
        "wedgehi_bd": _blockdiag(np.ascontiguousarray(whi.astype(npmm))),
        "wm_bd": _blockdiag(np.asarray(inputs["Wm"], np.float32).T),
        "wk_bd": _blockdiag(np.asarray(inputs["Wk"], np.float32).T),
        "wselfm_bd": _blockdiag(np.asarray(inputs["Wself_m"], np.float32).T),
        "wselfk_bd": _blockdiag(np.asarray(inputs["Wself_k"], np.float32).T),
        "wneighm_bd": _blockdiag(np.asarray(inputs["Wneigh_m"], np.float32).T / 64.0),
        "wneighk_bd": _blockdiag(np.asarray(inputs["Wneigh_k"], np.float32).T / 64.0),
        "eye64": np.eye(64).astype(npmm),
    }
    if split:
        consts["wedgelo_bd"] = _blockdiag(np.ascontiguousarray(wlo))

    in_maps = []
    for c in range(NCORES):
        sl = slice(c * BPC, (c + 1) * BPC)
        m = {"xhi": np.ascontiguousarray(xhi[sl]),
             "zmt": np.ascontiguousarray(zmt[sl]),
             "zkt": np.ascontiguousarray(zkt[sl])}
        if split:
            m["xlo"] = np.ascontiguousarray(xlo[sl])
        m.update(consts)
        in_maps.append(m)
    return in_maps


def _postprocess(results):
    edges_t = np.concatenate([r["edges_t"] for r in results], axis=0)
    zmup_t = np.concatenate([r["zmup_t"] for r in results], axis=0)
    zkup_t = np.concatenate([r["zkup_t"] for r in results], axis=0)
    z_mk_updated = np.ascontiguousarray(edges_t.transpose(0, 2, 1)).reshape(BS, M * K, FOUT)
    z_m_updated = np.ascontiguousarray(zmup_t.transpose(0, 2, 1))
    z_k_updated = np.ascontiguousarray(zkup_t.transpose(0, 2, 1))
    return z_mk_updated, z_m_updated, z_k_updated


def run_on_hw(in_maps, trace=False, **kwargs):
    from concourse.bass_utils import run_bass_kernel_spmd

    if "nc" not in _CACHE:
        _CACHE["nc"] = _build_nc()
    res = run_bass_kernel_spmd(
        _CACHE["nc"], in_maps, core_ids=list(range(NCORES)), trace=trace, **kwargs
    )
    return res


def kernel(**inputs):
    in_maps = _host_prep(inputs)
    res = run_on_hw(in_maps)
    return _postprocess(res.results)


# revision 25
# speedup vs baseline: 1.1653x; 1.0341x over previous
"""Trainium2 Bass kernel for the GNN message-passing layer.

Strategy (pure data-parallel over batch, 8 NeuronCores, 16 batches/core):
  - Host pre-transposes activations to feature-major layout so the device
    does zero on-chip transposes: all compute runs in [feature, element]
    layout with 2 batches packed on the 128 SBUF partitions.
  - Edge update: the (M,K)-grid broadcast-add of the per-m / per-k node
    terms is folded into the TensorE pass as PSUM-accumulated matmuls
    against a 0/1 indicator matrix, so VectorE never touches it.
  - Main matmuls run as bf16 hi/lo split (x = xhi + xlo, W = Whi + Wlo;
    keep Whi*xhi + Whi*xlo + Wlo*xhi): same DMA bytes as fp32, 1 cyc/row
    on the PE, ~1e-6 relative error. (MM_MODE="f32r" trades accuracy
    ~1.6e-4 for ~2x less PE work; "f32" is the 4 cyc/row exact path.)
  - leaky_relu runs on ScalarE (PSUM->SBUF, one pass).
  - mean-over-k runs on VectorE (one strided reduce per pair);
    mean-over-m runs on GpSimd as a log2 tree of tensor adds.
  - Node updates are tiny block-diagonal fp32 matmuls (mean 1/64 folded
    into the host-prepared Wneigh weights).
  - Loads issue on the SP HWDGE queue, stores on the ACT HWDGE queue so
    stores never head-of-line-block the next pair's loads.
"""
import sys

import numpy as np

sys.path.insert(0, "/opt/trn_rl_repo")

BS, M, K, FIN, FOUT, E = 128, 64, 64, 64, 64, 4096
NCORES = 8
BPC = BS // NCORES          # batches per core
PAIRS = BPC // 2            # 2 batches packed per 128 partitions
NEG_SLOPE = 0.01
WAVE = 1024                 # psum wave width (2 banks)
WAVES = E // WAVE

MM_MODE = "bf16split"       # "bf16split" | "f32r" | "f32"
NK_ON_GPSIMD = False        # Sum-over-m tree on GpSimd (else VectorE)

_CACHE = {}


def _build_nc(mm_mode=MM_MODE, nk_on_gpsimd=NK_ON_GPSIMD):
    import concourse.bacc as bacc
    import concourse.tile as tile
    from concourse import mybir
    from contextlib import ExitStack

    f32 = mybir.dt.float32
    bf16 = mybir.dt.bfloat16
    fmm = {"bf16split": bf16, "f32r": mybir.dt.float32r, "f32": f32}[mm_mode]
    split = mm_mode == "bf16split"
    Lrelu = mybir.ActivationFunctionType.Lrelu
    add = mybir.AluOpType.add
    X = mybir.AxisListType.X

    nc = bacc.Bacc("TRN2", target_bir_lowering=False, debug=False)

    # DRAM I/O (per core shard)
    xhi_d = nc.dram_tensor("xhi", [BPC, FIN, E], fmm, kind="ExternalInput").ap()
    if split:
        xlo_d = nc.dram_tensor("xlo", [BPC, FIN, E], fmm, kind="ExternalInput").ap()
    zmt_d = nc.dram_tensor("zmt", [BPC, FIN, M], f32, kind="ExternalInput").ap()
    zkt_d = nc.dram_tensor("zkt", [BPC, FIN, K], f32, kind="ExternalInput").ap()
    whi_d = nc.dram_tensor("wedgehi_bd", [128, 128], fmm, kind="ExternalInput").ap()
    if split:
        wlo_d = nc.dram_tensor("wedgelo_bd", [128, 128], fmm, kind="ExternalInput").ap()
    wm_d = nc.dram_tensor("wm_bd", [128, 128], f32, kind="ExternalInput").ap()
    wk_d = nc.dram_tensor("wk_bd", [128, 128], f32, kind="ExternalInput").ap()
    wsm_d = nc.dram_tensor("wselfm_bd", [128, 128], f32, kind="ExternalInput").ap()
    wsk_d = nc.dram_tensor("wselfk_bd", [128, 128], f32, kind="ExternalInput").ap()
    wnm_d = nc.dram_tensor("wneighm_bd", [128, 128], f32, kind="ExternalInput").ap()
    wnk_d = nc.dram_tensor("wneighk_bd", [128, 128], f32, kind="ExternalInput").ap()
    eye_d = nc.dram_tensor("eye64", [64, 64], fmm, kind="ExternalInput").ap()

    edges_o = nc.dram_tensor("edges_t", [BPC, FOUT, E], f32, kind="ExternalOutput").ap()
    zmup_o = nc.dram_tensor("zmup_t", [BPC, FOUT, M], f32, kind="ExternalOutput").ap()
    zkup_o = nc.dram_tensor("zkup_t", [BPC, FOUT, K], f32, kind="ExternalOutput").ap()

    ld = nc.sync          # load queue (SP HWDGE)
    st = nc.scalar        # store queue (ACT HWDGE)

    with tile.TileContext(nc) as tc, ExitStack() as ctx:
        consts = ctx.enter_context(tc.tile_pool(name="consts", bufs=1))
        xts = ctx.enter_context(tc.tile_pool(name="xts", bufs=3))
        edges_pool = ctx.enter_context(tc.tile_pool(name="edges", bufs=3))
        small_in = ctx.enter_context(tc.tile_pool(name="small_in", bufs=3))
        work = ctx.enter_context(tc.tile_pool(name="work", bufs=3))
        psum_main = ctx.enter_context(tc.tile_pool(name="psmain", bufs=2, space="PSUM"))
        psum_wz = ctx.enter_context(tc.tile_pool(name="pswz", bufs=2, space="PSUM"))
        psum_node = ctx.enter_context(tc.tile_pool(name="psnode", bufs=2, space="PSUM"))

        # ---- constants ----
        def load_const(name, ap_d, shape, dt=f32):
            t = consts.tile(shape, dt, tag=name)
            ld.dma_start(out=t[:], in_=ap_d)
            return t

        whi_sb = load_const("whi", whi_d, [128, 128], dt=fmm)
        if split:
            wlo_sb = load_const("wlo", wlo_d, [128, 128], dt=fmm)
        wm_sb = load_const("wm", wm_d, [128, 128])
        wk_sb = load_const("wk", wk_d, [128, 128])
        wsm_sb = load_const("wsm", wsm_d, [128, 128])
        wsk_sb = load_const("wsk", wsk_d, [128, 128])
        wnm_sb = load_const("wnm", wnm_d, [128, 128])
        wnk_sb = load_const("wnk", wnk_d, [128, 128])
        eye_sb = load_const("eye", eye_d, [64, 64], dt=fmm)

        # indicator [128, E]: rows 0-63 select m(i)=i//64, rows 64-127 select k(i)=i%64
        ind_sb = consts.tile([128, E], fmm, tag="ind")
        nc.vector.tensor_copy(
            ind_sb[0:64, :].rearrange("p (m k) -> p m k", k=K),
            eye_sb[:, :, None].to_broadcast([64, M, K]),
        )
        nc.vector.tensor_copy(
            ind_sb[64:128, :].rearrange("p (m k) -> p m k", k=K),
            eye_sb[:, None, :].to_broadcast([64, M, K]),
        )

        def node_updates(prev):
            """Deferred node-update block for a finished pair (runs while the
            next pair's waves occupy the PE, so the sum_nk dependency never
            head-of-line-blocks the wave matmuls)."""
            b0, zmt_pair, zkt_pair, sum_nm, sum_nk = prev
            node_m_ps = psum_node.tile([128, M], f32, tag="nd")
            nc.tensor.matmul(node_m_ps[:], wsm_sb[:], zmt_pair[:], start=True, stop=False)
            nc.tensor.matmul(node_m_ps[:], wnm_sb[:], sum_nm[:], start=False, stop=True)
            node_k_ps = psum_node.tile([128, K], f32, tag="nd")
            nc.tensor.matmul(node_k_ps[:], wsk_sb[:], zkt_pair[:], start=True, stop=False)
            nc.tensor.matmul(node_k_ps[:], wnk_sb[:], sum_nk[:], start=False, stop=True)
            node_m_sb = work.tile([128, M], f32, tag="node_m_sb")
            nc.scalar.activation(out=node_m_sb[:], in_=node_m_ps[:], func=Lrelu, alpha=NEG_SLOPE)
            node_k_sb = work.tile([128, K], f32, tag="node_k_sb")
            nc.scalar.activation(out=node_k_sb[:], in_=node_k_ps[:], func=Lrelu, alpha=NEG_SLOPE)
            st.dma_start(out=zmup_o[b0:b0 + 2].rearrange("b o m -> (b o) m"), in_=node_m_sb[:])
            st.dma_start(out=zkup_o[b0:b0 + 2].rearrange("b o k -> (b o) k"), in_=node_k_sb[:])

        prev_pair = None
        for p in range(PAIRS):
            b0 = 2 * p
            xhi_pair = xts.tile([128, E], fmm, tag="xhi_pair")
            ld.dma_start(out=xhi_pair[:], in_=xhi_d[b0:b0 + 2].rearrange("b f e -> (b f) e"))
            if split:
                xlo_pair = xts.tile([128, E], fmm, tag="xlo_pair")
                ld.dma_start(out=xlo_pair[:], in_=xlo_d[b0:b0 + 2].rearrange("b f e -> (b f) e"))
            zmt_pair = small_in.tile([128, M], f32, tag="zmt_pair")
            ld.dma_start(out=zmt_pair[:], in_=zmt_d[b0:b0 + 2].rearrange("b f m -> (b f) m"))
            zkt_pair = small_in.tile([128, K], f32, tag="zkt_pair")
            ld.dma_start(out=zkt_pair[:], in_=zkt_d[b0:b0 + 2].rearrange("b f k -> (b f) k"))

            # Wz_m / Wz_k in [node, (b, o)] layout -> stacked as lhsT for the S-matmul
            wzm_ps = psum_wz.tile([64, 128], f32, tag="wz")
            nc.tensor.matmul(wzm_ps[:], zmt_pair[:], wm_sb[:], start=True, stop=True)
            wzk_ps = psum_wz.tile([64, 128], f32, tag="wz")
            nc.tensor.matmul(wzk_ps[:], zkt_pair[:], wk_sb[:], start=True, stop=True)
            s_hi = work.tile([128, 128], fmm, tag="s_hi")
            nc.scalar.copy(out=s_hi[0:64, :], in_=wzm_ps[:])
            nc.scalar.copy(out=s_hi[64:128, :], in_=wzk_ps[:])
            if split:
                s_lo = work.tile([128, 128], fmm, tag="s_lo")
                nc.vector.tensor_tensor(
                    s_lo[0:64, :], wzm_ps[:], s_hi[0:64, :], mybir.AluOpType.subtract)
                nc.vector.tensor_tensor(
                    s_lo[64:128, :], wzk_ps[:], s_hi[64:128, :], mybir.AluOpType.subtract)

            edges_sb = edges_pool.tile([128, E], f32, tag="edges_sb")
            sum_nm = work.tile([128, M], f32, tag="sum_nm")
            nk_f01 = work.tile([128, WAVE], f32, tag="nk_f01")
            nk_f23 = work.tile([128, WAVE], f32, tag="nk_f23")

            for w in range(WAVES):
                ps = psum_main.tile([128, WAVE], f32, tag="ps")

                # Quadrant-tiled matmuls: per 512-col chunk the S terms run as a
                # row-split concurrent pair (K=64 each) and each main term as a
                # batch-split concurrent pair on disjoint (row, col) quadrants,
                # so LDWEIGHTS of one quadrant hides under the other's MATMUL.
                for h in range(2):
                    pc = ps[:, h * 512:(h + 1) * 512]
                    sl = slice(w * WAVE + h * 512, w * WAVE + (h + 1) * 512)

                    # Full-K block-diagonal matmuls (5 per chunk), one
                    # accumulation group per 512-col psum bank.
                    s_terms = [s_hi, s_lo] if split else [s_hi]
                    main_terms = [(whi_sb, xhi_pair)]
                    if split:
                        main_terms += [(whi_sb, xlo_pair), (wlo_sb, xhi_pair)]
                    for si, s_sb in enumerate(s_terms):
                        nc.tensor.matmul(pc, s_sb[:], ind_sb[:, sl],
                                         start=(si == 0), stop=False)
                    for ti, (wsb, xsb) in enumerate(main_terms):
                        nc.tensor.matmul(pc, wsb[:], xsb[:, sl],
                                         start=False, stop=(ti == len(main_terms) - 1))

                wsl = slice(w * WAVE, (w + 1) * WAVE)
                nc.scalar.activation(
                    out=edges_sb[:, wsl], in_=ps[:], func=Lrelu, alpha=NEG_SLOPE,
                )
                # incremental reductions per wave (hidden under the next
                # wave's matmuls)
                mpw = WAVE // K
                nc.vector.tensor_reduce(
                    out=sum_nm[:, w * mpw:(w + 1) * mpw],
                    in_=edges_sb[:, wsl].rearrange("p (m k) -> p m k", k=K),
                    axis=X, op=add,
                )
                # fold waves pairwise on GpSimd for the sum-over-m; only a
                # short strided reduce is left for VectorE at the tail
                if w == 1:
                    nc.gpsimd.tensor_tensor(nk_f01[:], edges_sb[:, 0:WAVE],
                                            edges_sb[:, WAVE:2 * WAVE], add)
                elif w == 3:
                    nc.gpsimd.tensor_tensor(nk_f23[:], edges_sb[:, 2 * WAVE:3 * WAVE],
                                            edges_sb[:, 3 * WAVE:4 * WAVE], add)
                    nc.gpsimd.tensor_tensor(nk_f01[:], nk_f01[:], nk_f23[:], add)
                # the previous pair's node updates slot in after wave 0 so
                # they sit behind only one wave group in the PE queue
                if w == 0 and prev_pair is not None:
                    node_updates(prev_pair)

            sum_nk = work.tile([128, K], f32, tag="sum_nk")
            nc.vector.tensor_reduce(
                out=sum_nk[:],
                in_=nk_f01[:].rearrange("p (m k) -> p k m", k=K),
                axis=X, op=add,
            )

            st.dma_start(
                out=edges_o[b0:b0 + 2].rearrange("b o e -> (b o) e"), in_=edges_sb[:]
            )
            prev_pair = (b0, zmt_pair, zkt_pair, sum_nm, sum_nk)

        node_updates(prev_pair)

    nc.compile()
    return nc


def _blockdiag(w):
    out = np.zeros((128, 128), w.dtype)
    out[:64, :64] = w
    out[64:, 64:] = w
    return out


def _host_prep(inputs, mm_mode=MM_MODE):
    import ml_dtypes

    bf16 = ml_dtypes.bfloat16
    split = mm_mode == "bf16split"
    npmm = bf16 if split else np.float32

    z_mk = np.asarray(inputs["z_mk"], np.float32)
    z_m = np.asarray(inputs["z_m"], np.float32)
    z_k = np.asarray(inputs["z_k"], np.float32)
    xt = np.ascontiguousarray(z_mk.transpose(0, 2, 1))
    zmt = np.ascontiguousarray(z_m.transpose(0, 2, 1))
    zkt = np.ascontiguousarray(z_k.transpose(0, 2, 1))

    wedge_t = np.asarray(inputs["Wedge"], np.float32).T
    if split:
        xhi = xt.astype(bf16)
        xlo = (xt - xhi.astype(np.float32)).astype(bf16)
        whi = wedge_t.astype(bf16)
        wlo = (wedge_t - whi.astype(np.float32)).astype(bf16)
    else:
        xhi, xlo = xt, None
        whi, wlo = wedge_t, None

    consts = {
        "wedgehi_bd": _blockdiag(np.ascontiguousarray(whi.astype(npmm))),
        "wm_bd": _blockdiag(np.asarray(inputs["Wm"], np.float32).T),
        "wk_bd": _blockdiag(np.asarray(inputs["Wk"], np.float32).T),
        "wselfm_bd": _blockdiag(np.asarray(inputs["Wself_m"], np.float32).T),
        "wselfk_bd": _blockdiag(np.asarray(inputs["Wself_k"], np.float32).T),
        "wneighm_bd": _blockdiag(np.asarray(inputs["Wneigh_m"], np.float32).T / 64.0),
        "wneighk_bd": _blockdiag(np.asarray(inputs["Wneigh_k"], np.float32).T / 64.0),
        "eye64": np.eye(64).astype(npmm),
    }
    if split:
        consts["wedgelo_bd"] = _blockdiag(np.ascontiguousarray(wlo))

    in_maps = []
    for c in range(NCORES):
        sl = slice(c * BPC, (c + 1) * BPC)
        m = {"xhi": np.ascontiguousarray(xhi[sl]),
             "zmt": np.ascontiguousarray(zmt[sl]),
             "zkt": np.ascontiguousarray(zkt[sl])}
        if split:
            m["xlo"] = np.ascontiguousarray(xlo[sl])
        m.update(consts)
        in_maps.append(m)
    return in_maps


def _postprocess(results):
    edges_t = np.concatenate([r["edges_t"] for r in results], axis=0)
    zmup_t = np.concatenate([r["zmup_t"] for r in results], axis=0)
    zkup_t = np.concatenate([r["zkup_t"] for r in results], axis=0)
    z_mk_updated = np.ascontiguousarray(edges_t.transpose(0, 2, 1)).reshape(BS, M * K, FOUT)
    z_m_updated = np.ascontiguousarray(zmup_t.transpose(0, 2, 1))
    z_k_updated = np.ascontiguousarray(zkup_t.transpose(0, 2, 1))
    return z_mk_updated, z_m_updated, z_k_updated


def run_on_hw(in_maps, trace=False, **kwargs):
    from concourse.bass_utils import run_bass_kernel_spmd

    if "nc" not in _CACHE:
        _CACHE["nc"] = _build_nc()
    res = run_bass_kernel_spmd(
        _CACHE["nc"], in_maps, core_ids=list(range(NCORES)), trace=trace, **kwargs
    )
    return res


def kernel(**inputs):
    in_maps = _host_prep(inputs)
    res = run_on_hw(in_maps)
    return _postprocess(res.results)


# revision 27
# speedup vs baseline: 1.2557x; 1.0775x over previous
"""Trainium2 Bass kernel for the GNN message-passing layer.

Strategy (pure data-parallel over batch, 8 NeuronCores, 16 batches/core):
  - Host pre-transposes activations to feature-major layout so the device
    does zero on-chip transposes: all compute runs in [feature, element]
    layout with 2 batches packed on the 128 SBUF partitions.
  - Edge update: the (M,K)-grid broadcast-add of the per-m / per-k node
    terms is folded into the TensorE pass as PSUM-accumulated matmuls
    against a 0/1 indicator matrix, so VectorE never touches it.
  - Main matmuls run as bf16 hi/lo split (x = xhi + xlo, W = Whi + Wlo;
    keep Whi*xhi + Whi*xlo + Wlo*xhi): same DMA bytes as fp32, 1 cyc/row
    on the PE, ~1e-6 relative error. (MM_MODE="f32r" trades accuracy
    ~1.6e-4 for ~2x less PE work; "f32" is the 4 cyc/row exact path.)
  - leaky_relu runs on ScalarE (PSUM->SBUF, one pass).
  - mean-over-k runs on VectorE (one strided reduce per pair);
    mean-over-m runs on GpSimd as a log2 tree of tensor adds.
  - Node updates are tiny block-diagonal fp32 matmuls (mean 1/64 folded
    into the host-prepared Wneigh weights).
  - Loads issue on the SP HWDGE queue, stores on the ACT HWDGE queue so
    stores never head-of-line-block the next pair's loads.
"""
import sys

import numpy as np

sys.path.insert(0, "/opt/trn_rl_repo")

BS, M, K, FIN, FOUT, E = 128, 64, 64, 64, 64, 4096
NCORES = 8
BPC = BS // NCORES          # batches per core
PAIRS = BPC // 2            # 2 batches packed per 128 partitions
NEG_SLOPE = 0.01
WAVE = 1024                 # psum wave width (2 banks)
WAVES = E // WAVE

MM_MODE = "bf16split"       # "bf16split" | "f32r" | "f32"
NK_ON_GPSIMD = False        # Sum-over-m tree on GpSimd (else VectorE)

_CACHE = {}


def _build_nc(mm_mode=MM_MODE, nk_on_gpsimd=NK_ON_GPSIMD):
    import concourse.bacc as bacc
    import concourse.tile as tile
    from concourse import mybir
    from contextlib import ExitStack

    f32 = mybir.dt.float32
    bf16 = mybir.dt.bfloat16
    fmm = {"bf16split": bf16, "f32r": mybir.dt.float32r, "f32": f32}[mm_mode]
    split = mm_mode == "bf16split"
    Lrelu = mybir.ActivationFunctionType.Lrelu
    add = mybir.AluOpType.add
    X = mybir.AxisListType.X

    nc = bacc.Bacc("TRN2", target_bir_lowering=False, debug=False)

    # DRAM I/O (per core shard)
    xhi_d = nc.dram_tensor("xhi", [BPC, FIN, E], fmm, kind="ExternalInput").ap()
    if split:
        xlo_d = nc.dram_tensor("xlo", [BPC, FIN, E], fmm, kind="ExternalInput").ap()
    zmt_d = nc.dram_tensor("zmt", [BPC, FIN, M], f32, kind="ExternalInput").ap()
    zkt_d = nc.dram_tensor("zkt", [BPC, FIN, K], f32, kind="ExternalInput").ap()
    whi_d = nc.dram_tensor("wedgehi_bd", [128, 128], fmm, kind="ExternalInput").ap()
    if split:
        wlo_d = nc.dram_tensor("wedgelo_bd", [128, 128], fmm, kind="ExternalInput").ap()
    wm_d = nc.dram_tensor("wm_bd", [128, 128], f32, kind="ExternalInput").ap()
    wk_d = nc.dram_tensor("wk_bd", [128, 128], f32, kind="ExternalInput").ap()
    wsm_d = nc.dram_tensor("wselfm_bd", [128, 128], f32, kind="ExternalInput").ap()
    wsk_d = nc.dram_tensor("wselfk_bd", [128, 128], f32, kind="ExternalInput").ap()
    wnm_d = nc.dram_tensor("wneighm_bd", [128, 128], f32, kind="ExternalInput").ap()
    wnk_d = nc.dram_tensor("wneighk_bd", [128, 128], f32, kind="ExternalInput").ap()
    eye_d = nc.dram_tensor("eye64", [64, 64], fmm, kind="ExternalInput").ap()

    edges_o = nc.dram_tensor("edges_t", [BPC, FOUT, E], f32, kind="ExternalOutput").ap()
    zmup_o = nc.dram_tensor("zmup_t", [BPC, FOUT, M], f32, kind="ExternalOutput").ap()
    zkup_o = nc.dram_tensor("zkup_t", [BPC, FOUT, K], f32, kind="ExternalOutput").ap()

    ld = nc.sync          # load queue (SP HWDGE)
    st = nc.scalar        # store queue (ACT HWDGE)

    with tile.TileContext(nc) as tc, ExitStack() as ctx:
        consts = ctx.enter_context(tc.tile_pool(name="consts", bufs=1))
        xts = ctx.enter_context(tc.tile_pool(name="xts", bufs=3))
        edges_pool = ctx.enter_context(tc.tile_pool(name="edges", bufs=3))
        small_in = ctx.enter_context(tc.tile_pool(name="small_in", bufs=3))
        work = ctx.enter_context(tc.tile_pool(name="work", bufs=3))
        psum_main = ctx.enter_context(tc.tile_pool(name="psmain", bufs=2, space="PSUM"))
        psum_wz = ctx.enter_context(tc.tile_pool(name="pswz", bufs=2, space="PSUM"))
        psum_node = ctx.enter_context(tc.tile_pool(name="psnode", bufs=2, space="PSUM"))

        # ---- constants ----
        def load_const(name, ap_d, shape, dt=f32):
            t = consts.tile(shape, dt, tag=name)
            ld.dma_start(out=t[:], in_=ap_d)
            return t

        whi_sb = load_const("whi", whi_d, [128, 128], dt=fmm)
        if split:
            wlo_sb = load_const("wlo", wlo_d, [128, 128], dt=fmm)
        wm_sb = load_const("wm", wm_d, [128, 128])
        wk_sb = load_const("wk", wk_d, [128, 128])
        wsm_sb = load_const("wsm", wsm_d, [128, 128])
        wsk_sb = load_const("wsk", wsk_d, [128, 128])
        wnm_sb = load_const("wnm", wnm_d, [128, 128])
        wnk_sb = load_const("wnk", wnk_d, [128, 128])
        eye_sb = load_const("eye", eye_d, [64, 64], dt=fmm)

        # indicator [128, E]: rows 0-63 select m(i)=i//64, rows 64-127 select k(i)=i%64
        ind_sb = consts.tile([128, E], fmm, tag="ind")
        nc.vector.tensor_copy(
            ind_sb[0:64, :].rearrange("p (m k) -> p m k", k=K),
            eye_sb[:, :, None].to_broadcast([64, M, K]),
        )
        nc.vector.tensor_copy(
            ind_sb[64:128, :].rearrange("p (m k) -> p m k", k=K),
            eye_sb[:, None, :].to_broadcast([64, M, K]),
        )

        def node_updates(prev):
            """Deferred node-update block for a finished pair (runs while the
            next pair's waves occupy the PE, so the sum_nk dependency never
            head-of-line-blocks the wave matmuls)."""
            b0, zmt_pair, zkt_pair, sum_nm, sum_nk = prev
            node_m_ps = psum_node.tile([128, M], f32, tag="nd")
            nc.tensor.matmul(node_m_ps[:], wsm_sb[:], zmt_pair[:], start=True, stop=False)
            nc.tensor.matmul(node_m_ps[:], wnm_sb[:], sum_nm[:], start=False, stop=True)
            node_k_ps = psum_node.tile([128, K], f32, tag="nd")
            nc.tensor.matmul(node_k_ps[:], wsk_sb[:], zkt_pair[:], start=True, stop=False)
            nc.tensor.matmul(node_k_ps[:], wnk_sb[:], sum_nk[:], start=False, stop=True)
            node_m_sb = work.tile([128, M], f32, tag="node_m_sb")
            nc.scalar.activation(out=node_m_sb[:], in_=node_m_ps[:], func=Lrelu, alpha=NEG_SLOPE)
            node_k_sb = work.tile([128, K], f32, tag="node_k_sb")
            nc.scalar.activation(out=node_k_sb[:], in_=node_k_ps[:], func=Lrelu, alpha=NEG_SLOPE)
            st.dma_start(out=zmup_o[b0:b0 + 2].rearrange("b o m -> (b o) m"), in_=node_m_sb[:])
            st.dma_start(out=zkup_o[b0:b0 + 2].rearrange("b o k -> (b o) k"), in_=node_k_sb[:])

        prev_pair = None
        for p in range(PAIRS):
            b0 = 2 * p
            # small loads first: they gate the first PE ops of the pair and
            # must not queue behind the 2MB x transfers
            zmt_pair = small_in.tile([128, M], f32, tag="zmt_pair")
            ld.dma_start(out=zmt_pair[:], in_=zmt_d[b0:b0 + 2].rearrange("b f m -> (b f) m"))
            zkt_pair = small_in.tile([128, K], f32, tag="zkt_pair")
            ld.dma_start(out=zkt_pair[:], in_=zkt_d[b0:b0 + 2].rearrange("b f k -> (b f) k"))
            xhi_pair = xts.tile([128, E], fmm, tag="xhi_pair")
            ld.dma_start(out=xhi_pair[:], in_=xhi_d[b0:b0 + 2].rearrange("b f e -> (b f) e"))
            if split:
                xlo_pair = xts.tile([128, E], fmm, tag="xlo_pair")
                ld.dma_start(out=xlo_pair[:], in_=xlo_d[b0:b0 + 2].rearrange("b f e -> (b f) e"))

            # Wz_m / Wz_k in [node, (b, o)] layout -> stacked as lhsT for the S-matmul
            wzm_ps = psum_wz.tile([64, 128], f32, tag="wz")
            nc.tensor.matmul(wzm_ps[:], zmt_pair[:], wm_sb[:], start=True, stop=True)
            wzk_ps = psum_wz.tile([64, 128], f32, tag="wz")
            nc.tensor.matmul(wzk_ps[:], zkt_pair[:], wk_sb[:], start=True, stop=True)
            s_hi = work.tile([128, 128], fmm, tag="s_hi")
            nc.scalar.copy(out=s_hi[0:64, :], in_=wzm_ps[:])
            nc.scalar.copy(out=s_hi[64:128, :], in_=wzk_ps[:])
            if split:
                s_lo = work.tile([128, 128], fmm, tag="s_lo")
                nc.vector.tensor_tensor(
                    s_lo[0:64, :], wzm_ps[:], s_hi[0:64, :], mybir.AluOpType.subtract)
                nc.vector.tensor_tensor(
                    s_lo[64:128, :], wzk_ps[:], s_hi[64:128, :], mybir.AluOpType.subtract)

            edges_sb = edges_pool.tile([128, E], f32, tag="edges_sb")
            sum_nm = work.tile([128, M], f32, tag="sum_nm")
            nk_f01 = work.tile([128, WAVE], f32, tag="nk_f01")
            nk_f23 = work.tile([128, WAVE], f32, tag="nk_f23")

            for w in range(WAVES):
                ps = psum_main.tile([128, WAVE], f32, tag="ps")

                # Quadrant-tiled matmuls: per 512-col chunk the S terms run as a
                # row-split concurrent pair (K=64 each) and each main term as a
                # batch-split concurrent pair on disjoint (row, col) quadrants,
                # so LDWEIGHTS of one quadrant hides under the other's MATMUL.
                for h in range(2):
                    pc = ps[:, h * 512:(h + 1) * 512]
                    sl = slice(w * WAVE + h * 512, w * WAVE + (h + 1) * 512)

                    # Full-K block-diagonal matmuls (5 per chunk), one
                    # accumulation group per 512-col psum bank. Main terms go
                    # first: they depend only on prefetched DMA, while the S
                    # terms wait on this pair's Wz copy chain.
                    s_terms = [s_hi, s_lo] if split else [s_hi]
                    main_terms = [(whi_sb, xhi_pair)]
                    if split:
                        main_terms += [(whi_sb, xlo_pair), (wlo_sb, xhi_pair)]
                    for ti, (wsb, xsb) in enumerate(main_terms):
                        nc.tensor.matmul(pc, wsb[:], xsb[:, sl],
                                         start=(ti == 0), stop=False)
                    for si, s_sb in enumerate(s_terms):
                        nc.tensor.matmul(pc, s_sb[:], ind_sb[:, sl],
                                         start=False, stop=(si == len(s_terms) - 1))

                wsl = slice(w * WAVE, (w + 1) * WAVE)
                nc.scalar.activation(
                    out=edges_sb[:, wsl], in_=ps[:], func=Lrelu, alpha=NEG_SLOPE,
                )
                # incremental reductions per wave (hidden under the next
                # wave's matmuls)
                mpw = WAVE // K
                nc.vector.tensor_reduce(
                    out=sum_nm[:, w * mpw:(w + 1) * mpw],
                    in_=edges_sb[:, wsl].rearrange("p (m k) -> p m k", k=K),
                    axis=X, op=add,
                )
                # fold waves pairwise on GpSimd for the sum-over-m; only a
                # short strided reduce is left for VectorE at the tail
                if w == 1:
                    nc.gpsimd.tensor_tensor(nk_f01[:], edges_sb[:, 0:WAVE],
                                            edges_sb[:, WAVE:2 * WAVE], add)
                elif w == 3:
                    nc.gpsimd.tensor_tensor(nk_f23[:], edges_sb[:, 2 * WAVE:3 * WAVE],
                                            edges_sb[:, 3 * WAVE:4 * WAVE], add)
                    nc.gpsimd.tensor_tensor(nk_f01[:], nk_f01[:], nk_f23[:], add)
                # the previous pair's node updates slot in after wave 0 so
                # they sit behind only one wave group in the PE queue
                if w == 0 and prev_pair is not None:
                    node_updates(prev_pair)

            sum_nk = work.tile([128, K], f32, tag="sum_nk")
            nc.vector.tensor_reduce(
                out=sum_nk[:],
                in_=nk_f01[:].rearrange("p (m k) -> p k m", k=K),
                axis=X, op=add,
            )

            st.dma_start(
                out=edges_o[b0:b0 + 2].rearrange("b o e -> (b o) e"), in_=edges_sb[:]
            )
            prev_pair = (b0, zmt_pair, zkt_pair, sum_nm, sum_nk)

        node_updates(prev_pair)

    nc.compile()
    return nc


def _blockdiag(w):
    out = np.zeros((128, 128), w.dtype)
    out[:64, :64] = w
    out[64:, 64:] = w
    return out


def _host_prep(inputs, mm_mode=MM_MODE):
    import ml_dtypes

    bf16 = ml_dtypes.bfloat16
    split = mm_mode == "bf16split"
    npmm = bf16 if split else np.float32

    z_mk = np.asarray(inputs["z_mk"], np.float32)
    z_m = np.asarray(inputs["z_m"], np.float32)
    z_k = np.asarray(inputs["z_k"], np.float32)
    xt = np.ascontiguousarray(z_mk.transpose(0, 2, 1))
    zmt = np.ascontiguousarray(z_m.transpose(0, 2, 1))
    zkt = np.ascontiguousarray(z_k.transpose(0, 2, 1))

    wedge_t = np.asarray(inputs["Wedge"], np.float32).T
    if split:
        xhi = xt.astype(bf16)
        xlo = (xt - xhi.astype(np.float32)).astype(bf16)
        whi = wedge_t.astype(bf16)
        wlo = (wedge_t - whi.astype(np.float32)).astype(bf16)
    else:
        xhi, xlo = xt, None
        whi, wlo = wedge_t, None

    consts = {
        "wedgehi_bd": _blockdiag(np.ascontiguousarray(whi.astype(npmm))),
        "wm_bd": _blockdiag(np.asarray(inputs["Wm"], np.float32).T),
        "wk_bd": _blockdiag(np.asarray(inputs["Wk"], np.float32).T),
        "wselfm_bd": _blockdiag(np.asarray(inputs["Wself_m"], np.float32).T),
        "wselfk_bd": _blockdiag(np.asarray(inputs["Wself_k"], np.float32).T),
        "wneighm_bd": _blockdiag(np.asarray(inputs["Wneigh_m"], np.float32).T / 64.0),
        "wneighk_bd": _blockdiag(np.asarray(inputs["Wneigh_k"], np.float32).T / 64.0),
        "eye64": np.eye(64).astype(npmm),
    }
    if split:
        consts["wedgelo_bd"] = _blockdiag(np.ascontiguousarray(wlo))

    in_maps = []
    for c in range(NCORES):
        sl = slice(c * BPC, (c + 1) * BPC)
        m = {"xhi": np.ascontiguousarray(xhi[sl]),
             "zmt": np.ascontiguousarray(zmt[sl]),
             "zkt": np.ascontiguousarray(zkt[sl])}
        if split:
            m["xlo"] = np.ascontiguousarray(xlo[sl])
        m.update(consts)
        in_maps.append(m)
    return in_maps


def _postprocess(results):
    edges_t = np.concatenate([r["edges_t"] for r in results], axis=0)
    zmup_t = np.concatenate([r["zmup_t"] for r in results], axis=0)
    zkup_t = np.concatenate([r["zkup_t"] for r in results], axis=0)
    z_mk_updated = np.ascontiguousarray(edges_t.transpose(0, 2, 1)).reshape(BS, M * K, FOUT)
    z_m_updated = np.ascontiguousarray(zmup_t.transpose(0, 2, 1))
    z_k_updated = np.ascontiguousarray(zkup_t.transpose(0, 2, 1))
    return z_mk_updated, z_m_updated, z_k_updated


def run_on_hw(in_maps, trace=False, **kwargs):
    from concourse.bass_utils import run_bass_kernel_spmd

    if "nc" not in _CACHE:
        _CACHE["nc"] = _build_nc()
    res = run_bass_kernel_spmd(
        _CACHE["nc"], in_maps, core_ids=list(range(NCORES)), trace=trace, **kwargs
    )
    return res


def kernel(**inputs):
    in_maps = _host_prep(inputs)
    res = run_on_hw(in_maps)
    return _postprocess(res.results)


# revision 29
# speedup vs baseline: 1.3920x; 1.1085x over previous
"""Trainium2 Bass kernel for the GNN message-passing layer.

Strategy (pure data-parallel over batch, 8 NeuronCores, 16 batches/core):
  - Host pre-transposes activations to feature-major layout so the device
    does zero on-chip transposes: all compute runs in [feature, element]
    layout with 2 batches packed on the 128 SBUF partitions.
  - Edge update: the (M,K)-grid broadcast-add of the per-m / per-k node
    terms is folded into the TensorE pass as PSUM-accumulated matmuls
    against a 0/1 indicator matrix, so VectorE never touches it.
  - Main matmuls run as bf16 hi/lo split (x = xhi + xlo, W = Whi + Wlo;
    keep Whi*xhi + Whi*xlo + Wlo*xhi): same DMA bytes as fp32, 1 cyc/row
    on the PE, ~1e-6 relative error. (MM_MODE="f32r" trades accuracy
    ~1.6e-4 for ~2x less PE work; "f32" is the 4 cyc/row exact path.)
  - leaky_relu runs on ScalarE (PSUM->SBUF, one pass).
  - mean-over-k runs on VectorE (one strided reduce per pair);
    mean-over-m runs on GpSimd as a log2 tree of tensor adds.
  - Node updates are tiny block-diagonal fp32 matmuls (mean 1/64 folded
    into the host-prepared Wneigh weights).
  - Loads issue on the SP HWDGE queue, stores on the ACT HWDGE queue so
    stores never head-of-line-block the next pair's loads.
"""
import sys

import numpy as np

sys.path.insert(0, "/opt/trn_rl_repo")

BS, M, K, FIN, FOUT, E = 128, 64, 64, 64, 64, 4096
NCORES = 8
BPC = BS // NCORES          # batches per core
PAIRS = BPC // 2            # 2 batches packed per 128 partitions
NEG_SLOPE = 0.01
WAVE = 1024                 # psum wave width (2 banks)
WAVES = E // WAVE

MM_MODE = "hybrid"          # "hybrid" | "bf16split" | "f32r" | "f32"
NK_ON_GPSIMD = False        # Sum-over-m tree on GpSimd (else VectorE)

_CACHE = {}


def _build_nc(mm_mode=MM_MODE, nk_on_gpsimd=NK_ON_GPSIMD):
    import concourse.bacc as bacc
    import concourse.tile as tile
    from concourse import mybir
    from contextlib import ExitStack

    f32 = mybir.dt.float32
    bf16 = mybir.dt.bfloat16
    f32r = mybir.dt.float32r
    fmm = {"bf16split": bf16, "hybrid": bf16, "f32r": f32r, "f32": f32}[mm_mode]
    split = mm_mode in ("bf16split", "hybrid")
    # hybrid: S term as a single f32r matmul (indicator is 0/1 = exact in
    # f32r; only the Wz operand is TF32-rounded, ~7e-5 relative on edges)
    s_split = mm_mode == "bf16split"
    s_dt = f32r if mm_mode == "hybrid" else fmm
    Lrelu = mybir.ActivationFunctionType.Lrelu
    add = mybir.AluOpType.add
    X = mybir.AxisListType.X

    nc = bacc.Bacc("TRN2", target_bir_lowering=False, debug=False)

    # DRAM I/O (per core shard)
    xhi_d = nc.dram_tensor("xhi", [BPC, FIN, E], fmm, kind="ExternalInput").ap()
    if split:
        xlo_d = nc.dram_tensor("xlo", [BPC, FIN, E], fmm, kind="ExternalInput").ap()
    zmt_d = nc.dram_tensor("zmt", [BPC, FIN, M], f32, kind="ExternalInput").ap()
    zkt_d = nc.dram_tensor("zkt", [BPC, FIN, K], f32, kind="ExternalInput").ap()
    whi_d = nc.dram_tensor("wedgehi_bd", [128, 128], fmm, kind="ExternalInput").ap()
    if split:
        wlo_d = nc.dram_tensor("wedgelo_bd", [128, 128], fmm, kind="ExternalInput").ap()
    wm_d = nc.dram_tensor("wm_bd", [128, 128], f32, kind="ExternalInput").ap()
    wk_d = nc.dram_tensor("wk_bd", [128, 128], f32, kind="ExternalInput").ap()
    wsm_d = nc.dram_tensor("wselfm_bd", [128, 128], f32, kind="ExternalInput").ap()
    wsk_d = nc.dram_tensor("wselfk_bd", [128, 128], f32, kind="ExternalInput").ap()
    wnm_d = nc.dram_tensor("wneighm_bd", [128, 128], f32, kind="ExternalInput").ap()
    wnk_d = nc.dram_tensor("wneighk_bd", [128, 128], f32, kind="ExternalInput").ap()
    eye_d = nc.dram_tensor("eye64", [64, 64], s_dt, kind="ExternalInput").ap()

    edges_o = nc.dram_tensor("edges_t", [BPC, FOUT, E], f32, kind="ExternalOutput").ap()
    zmup_o = nc.dram_tensor("zmup_t", [BPC, FOUT, M], f32, kind="ExternalOutput").ap()
    zkup_o = nc.dram_tensor("zkup_t", [BPC, FOUT, K], f32, kind="ExternalOutput").ap()

    ld = nc.sync          # load queue (SP HWDGE)
    st = nc.scalar        # store queue (ACT HWDGE)

    with tile.TileContext(nc) as tc, ExitStack() as ctx:
        consts = ctx.enter_context(tc.tile_pool(name="consts", bufs=1))
        xts = ctx.enter_context(tc.tile_pool(name="xts", bufs=3))
        edges_pool = ctx.enter_context(tc.tile_pool(name="edges", bufs=3))
        small_in = ctx.enter_context(tc.tile_pool(name="small_in", bufs=3))
        work = ctx.enter_context(tc.tile_pool(name="work", bufs=3))
        psum_main = ctx.enter_context(tc.tile_pool(name="psmain", bufs=2, space="PSUM"))
        psum_wz = ctx.enter_context(tc.tile_pool(name="pswz", bufs=2, space="PSUM"))
        psum_node = ctx.enter_context(tc.tile_pool(name="psnode", bufs=2, space="PSUM"))

        # ---- constants ----
        def load_const(name, ap_d, shape, dt=f32):
            t = consts.tile(shape, dt, tag=name)
            ld.dma_start(out=t[:], in_=ap_d)
            return t

        whi_sb = load_const("whi", whi_d, [128, 128], dt=fmm)
        if split:
            wlo_sb = load_const("wlo", wlo_d, [128, 128], dt=fmm)
        wm_sb = load_const("wm", wm_d, [128, 128])
        wk_sb = load_const("wk", wk_d, [128, 128])
        wsm_sb = load_const("wsm", wsm_d, [128, 128])
        wsk_sb = load_const("wsk", wsk_d, [128, 128])
        wnm_sb = load_const("wnm", wnm_d, [128, 128])
        wnk_sb = load_const("wnk", wnk_d, [128, 128])
        eye_sb = load_const("eye", eye_d, [64, 64], dt=s_dt)

        # indicator [128, E]: rows 0-63 select m(i)=i//64, rows 64-127 select k(i)=i%64
        ind_sb = consts.tile([128, E], s_dt, tag="ind")
        nc.vector.tensor_copy(
            ind_sb[0:64, :].rearrange("p (m k) -> p m k", k=K),
            eye_sb[:, :, None].to_broadcast([64, M, K]),
        )
        nc.vector.tensor_copy(
            ind_sb[64:128, :].rearrange("p (m k) -> p m k", k=K),
            eye_sb[:, None, :].to_broadcast([64, M, K]),
        )

        def node_updates(prev):
            """Deferred node-update block for a finished pair (runs while the
            next pair's waves occupy the PE, so the sum_nk dependency never
            head-of-line-blocks the wave matmuls)."""
            b0, zmt_pair, zkt_pair, sum_nm, sum_nk = prev
            node_m_ps = psum_node.tile([128, M], f32, tag="nd")
            nc.tensor.matmul(node_m_ps[:], wsm_sb[:], zmt_pair[:], start=True, stop=False)
            nc.tensor.matmul(node_m_ps[:], wnm_sb[:], sum_nm[:], start=False, stop=True)
            node_k_ps = psum_node.tile([128, K], f32, tag="nd")
            nc.tensor.matmul(node_k_ps[:], wsk_sb[:], zkt_pair[:], start=True, stop=False)
            nc.tensor.matmul(node_k_ps[:], wnk_sb[:], sum_nk[:], start=False, stop=True)
            node_m_sb = work.tile([128, M], f32, tag="node_m_sb")
            nc.scalar.activation(out=node_m_sb[:], in_=node_m_ps[:], func=Lrelu, alpha=NEG_SLOPE)
            node_k_sb = work.tile([128, K], f32, tag="node_k_sb")
            nc.scalar.activation(out=node_k_sb[:], in_=node_k_ps[:], func=Lrelu, alpha=NEG_SLOPE)
            st.dma_start(out=zmup_o[b0:b0 + 2].rearrange("b o m -> (b o) m"), in_=node_m_sb[:])
            st.dma_start(out=zkup_o[b0:b0 + 2].rearrange("b o k -> (b o) k"), in_=node_k_sb[:])

        prev_pair = None
        for p in range(PAIRS):
            b0 = 2 * p
            # small loads first: they gate the first PE ops of the pair and
            # must not queue behind the 2MB x transfers
            zmt_pair = small_in.tile([128, M], f32, tag="zmt_pair")
            ld.dma_start(out=zmt_pair[:], in_=zmt_d[b0:b0 + 2].rearrange("b f m -> (b f) m"))
            zkt_pair = small_in.tile([128, K], f32, tag="zkt_pair")
            ld.dma_start(out=zkt_pair[:], in_=zkt_d[b0:b0 + 2].rearrange("b f k -> (b f) k"))
            xhi_pair = xts.tile([128, E], fmm, tag="xhi_pair")
            ld.dma_start(out=xhi_pair[:], in_=xhi_d[b0:b0 + 2].rearrange("b f e -> (b f) e"))
            if split:
                xlo_pair = xts.tile([128, E], fmm, tag="xlo_pair")
                ld.dma_start(out=xlo_pair[:], in_=xlo_d[b0:b0 + 2].rearrange("b f e -> (b f) e"))

            # Wz_m / Wz_k in [node, (b, o)] layout -> stacked as lhsT for the S-matmul
            wzm_ps = psum_wz.tile([64, 128], f32, tag="wz")
            nc.tensor.matmul(wzm_ps[:], zmt_pair[:], wm_sb[:], start=True, stop=True)
            wzk_ps = psum_wz.tile([64, 128], f32, tag="wz")
            nc.tensor.matmul(wzk_ps[:], zkt_pair[:], wk_sb[:], start=True, stop=True)
            s_hi = work.tile([128, 128], s_dt, tag="s_hi")
            nc.scalar.copy(out=s_hi[0:64, :], in_=wzm_ps[:])
            nc.scalar.copy(out=s_hi[64:128, :], in_=wzk_ps[:])
            if s_split:
                s_lo = work.tile([128, 128], fmm, tag="s_lo")
                nc.vector.tensor_tensor(
                    s_lo[0:64, :], wzm_ps[:], s_hi[0:64, :], mybir.AluOpType.subtract)
                nc.vector.tensor_tensor(
                    s_lo[64:128, :], wzk_ps[:], s_hi[64:128, :], mybir.AluOpType.subtract)

            edges_sb = edges_pool.tile([128, E], f32, tag="edges_sb")
            sum_nm = work.tile([128, M], f32, tag="sum_nm")
            nk_f01 = work.tile([128, WAVE], f32, tag="nk_f01")
            nk_f23 = work.tile([128, WAVE], f32, tag="nk_f23")

            for w in range(WAVES):
                ps = psum_main.tile([128, WAVE], f32, tag="ps")

                # Quadrant-tiled matmuls: per 512-col chunk the S terms run as a
                # row-split concurrent pair (K=64 each) and each main term as a
                # batch-split concurrent pair on disjoint (row, col) quadrants,
                # so LDWEIGHTS of one quadrant hides under the other's MATMUL.
                for h in range(2):
                    pc = ps[:, h * 512:(h + 1) * 512]
                    sl = slice(w * WAVE + h * 512, w * WAVE + (h + 1) * 512)

                    # Full-K block-diagonal matmuls (5 per chunk), one
                    # accumulation group per 512-col psum bank. Main terms go
                    # first: they depend only on prefetched DMA, while the S
                    # terms wait on this pair's Wz copy chain.
                    s_terms = [s_hi, s_lo] if s_split else [s_hi]
                    main_terms = [(whi_sb, xhi_pair)]
                    if split:
                        main_terms += [(whi_sb, xlo_pair), (wlo_sb, xhi_pair)]
                    for ti, (wsb, xsb) in enumerate(main_terms):
                        nc.tensor.matmul(pc, wsb[:], xsb[:, sl],
                                         start=(ti == 0), stop=False)
                    for si, s_sb in enumerate(s_terms):
                        nc.tensor.matmul(pc, s_sb[:], ind_sb[:, sl],
                                         start=False, stop=(si == len(s_terms) - 1))

                wsl = slice(w * WAVE, (w + 1) * WAVE)
                nc.scalar.activation(
                    out=edges_sb[:, wsl], in_=ps[:], func=Lrelu, alpha=NEG_SLOPE,
                )
                # incremental reductions per wave (hidden under the next
                # wave's matmuls)
                mpw = WAVE // K
                nc.vector.tensor_reduce(
                    out=sum_nm[:, w * mpw:(w + 1) * mpw],
                    in_=edges_sb[:, wsl].rearrange("p (m k) -> p m k", k=K),
                    axis=X, op=add,
                )
                # fold waves pairwise on GpSimd for the sum-over-m; only a
                # short strided reduce is left for VectorE at the tail
                if w == 1:
                    nc.gpsimd.tensor_tensor(nk_f01[:], edges_sb[:, 0:WAVE],
                                            edges_sb[:, WAVE:2 * WAVE], add)
                elif w == 3:
                    nc.gpsimd.tensor_tensor(nk_f23[:], edges_sb[:, 2 * WAVE:3 * WAVE],
                                            edges_sb[:, 3 * WAVE:4 * WAVE], add)
                    nc.gpsimd.tensor_tensor(nk_f01[:], nk_f01[:], nk_f23[:], add)
                # the previous pair's node updates slot in after wave 0 so
                # they sit behind only one wave group in the PE queue
                if w == 0 and prev_pair is not None:
                    node_updates(prev_pair)

            sum_nk = work.tile([128, K], f32, tag="sum_nk")
            nc.vector.tensor_reduce(
                out=sum_nk[:],
                in_=nk_f01[:].rearrange("p (m k) -> p k m", k=K),
                axis=X, op=add,
            )

            st.dma_start(
                out=edges_o[b0:b0 + 2].rearrange("b o e -> (b o) e"), in_=edges_sb[:]
            )
            prev_pair = (b0, zmt_pair, zkt_pair, sum_nm, sum_nk)

        node_updates(prev_pair)

    nc.compile()
    return nc


def _blockdiag(w):
    out = np.zeros((128, 128), w.dtype)
    out[:64, :64] = w
    out[64:, 64:] = w
    return out


def _host_prep(inputs, mm_mode=MM_MODE):
    import ml_dtypes

    bf16 = ml_dtypes.bfloat16
    split = mm_mode in ("bf16split", "hybrid")
    npmm = bf16 if split else np.float32
    np_s = np.float32 if mm_mode in ("hybrid", "f32r", "f32") else npmm

    z_mk = np.asarray(inputs["z_mk"], np.float32)
    z_m = np.asarray(inputs["z_m"], np.float32)
    z_k = np.asarray(inputs["z_k"], np.float32)
    xt = np.ascontiguousarray(z_mk.transpose(0, 2, 1))
    zmt = np.ascontiguousarray(z_m.transpose(0, 2, 1))
    zkt = np.ascontiguousarray(z_k.transpose(0, 2, 1))

    wedge_t = np.asarray(inputs["Wedge"], np.float32).T
    if split:
        xhi = xt.astype(bf16)
        xlo = (xt - xhi.astype(np.float32)).astype(bf16)
        whi = wedge_t.astype(bf16)
        wlo = (wedge_t - whi.astype(np.float32)).astype(bf16)
    else:
        xhi, xlo = xt, None
        whi, wlo = wedge_t, None

    consts = {
        "wedgehi_bd": _blockdiag(np.ascontiguousarray(whi.astype(npmm))),
        "wm_bd": _blockdiag(np.asarray(inputs["Wm"], np.float32).T),
        "wk_bd": _blockdiag(np.asarray(inputs["Wk"], np.float32).T),
        "wselfm_bd": _blockdiag(np.asarray(inputs["Wself_m"], np.float32).T),
        "wselfk_bd": _blockdiag(np.asarray(inputs["Wself_k"], np.float32).T),
        "wneighm_bd": _blockdiag(np.asarray(inputs["Wneigh_m"], np.float32).T / 64.0),
        "wneighk_bd": _blockdiag(np.asarray(inputs["Wneigh_k"], np.float32).T / 64.0),
        "eye64": np.eye(64).astype(np_s),
    }
    if split:
        consts["wedgelo_bd"] = _blockdiag(np.ascontiguousarray(wlo))

    in_maps = []
    for c in range(NCORES):
        sl = slice(c * BPC, (c + 1) * BPC)
        m = {"xhi": np.ascontiguousarray(xhi[sl]),
             "zmt": np.ascontiguousarray(zmt[sl]),
             "zkt": np.ascontiguousarray(zkt[sl])}
        if split:
            m["xlo"] = np.ascontiguousarray(xlo[sl])
        m.update(consts)
        in_maps.append(m)
    return in_maps


def _postprocess(results):
    edges_t = np.concatenate([r["edges_t"] for r in results], axis=0)
    zmup_t = np.concatenate([r["zmup_t"] for r in results], axis=0)
    zkup_t = np.concatenate([r["zkup_t"] for r in results], axis=0)
    z_mk_updated = np.ascontiguousarray(edges_t.transpose(0, 2, 1)).reshape(BS, M * K, FOUT)
    z_m_updated = np.ascontiguousarray(zmup_t.transpose(0, 2, 1))
    z_k_updated = np.ascontiguousarray(zkup_t.transpose(0, 2, 1))
    return z_mk_updated, z_m_updated, z_k_updated


def run_on_hw(in_maps, trace=False, **kwargs):
    from concourse.bass_utils import run_bass_kernel_spmd

    if "nc" not in _CACHE:
        _CACHE["nc"] = _build_nc()
    res = run_bass_kernel_spmd(
        _CACHE["nc"], in_maps, core_ids=list(range(NCORES)), trace=trace, **kwargs
    )
    return res


def kernel(**inputs):
    in_maps = _host_prep(inputs)
    res = run_on_hw(in_maps)
    return _postprocess(res.results)
